# revision 11
# baseline (speedup 1.0000x reference)
"""Trainium2 Bass kernel for windowed ViT attention with decomposed relative
position bias (B=8, N=1024=32x32, C=768, 12 heads, head_dim 64).

Sharding: data-parallel over batch B across 8 NeuronCores (1 image per core).

Per-core algorithm (v2):
  - q/k computed in transposed layout qT/kT [d, n] (v in natural [n, d]) from
    host-pretransposed x and weights (f32r = fp32 bits, relaxed-precision PE
    mode); q-scale folded into the q rows of the qkv weight on the host.
  - rel-pos bias folded into the attention matmul by augmenting the
    contraction dim from 64 to exactly 128:
       S_T[k2, q] = sum_d kT[d,k2] qT[d,q]
                  + sum_i Ih[i,k2] rel_hT[i,q] + sum_j Iw[j,k2] rel_wT[j,q]
    with constant 0/1 indicator rows and Toeplitz-sliced rel tables.
  - The whole attention operand path (qaug, kasm, rel tables, exp output,
    vaug, atile, wp) is bf16: same PE column rate as f32r, half the SBUF
    footprint and DVE copy volume; adds ~1e-3 relative error (budget 2e-2).
  - softmax denominator rides as a ones-column appended to V (attnV out has
    65 rows; the extra row is free since PE cost is column-bound); exp is
    fused with PSUM evacuation on the scalar engine (ACT is the phase-D
    bottleneck at ~1.04us per [128,1024] chunk).
  - denominator -> reciprocal path: evacuate o_ps to SBUF, gpsimd
    partition_broadcast of the den row to 64 partitions, DVE reciprocal,
    DVE normalize into atile.  No DRAM round-trip, 2-stage pipeline.
  - s_ps PSUM pool has bufs=3 so the S matmuls run up to 2 chunks ahead of
    exp: ACT streams through head boundaries without stalling.
  - phase E (output projection) borrows s_ps PSUM slots (same tag) and its
    first 5 k-blocks per chunk are issued during the denominator drain of
    the last heads, keeping PE busy (and at full p-state) through the
    transition; the k=5 block (heads 10/11) finishes each chunk.
  - weights (wq/wv/wp/bias/ind/rel tables) are loaded into SBUF once,
    outside the rep loop; xt is re-DMA'd per rep into a persistent buffer
    right after its last phase-D reader, so back-to-back invocations never
    stall on input DMA.
"""

import sys

if "/opt/trn_rl_repo" not in sys.path:
    sys.path.insert(0, "/opt/trn_rl_repo")

import numpy as np

NUM_HEADS = 12
N_CTX = 1024
C_DIM = 768
HD = 64
HH = 32
NCORES = 8

_CACHE: dict = {}


def _build_nc(reps=1):
    import concourse.mybir as mybir
    import concourse.tile as tile
    from concourse import bacc
    from contextlib import ExitStack

    f32 = mybir.dt.float32
    f32r = mybir.dt.float32r
    bf16 = mybir.dt.bfloat16
    Exp = mybir.ActivationFunctionType.Exp

    nc = bacc.Bacc("TRN2", target_bir_lowering=False, debug=False)

    def mm(out, lhsT, rhs, **kw):
        nc.tensor.matmul(out, lhsT, rhs, **kw)

    xt = nc.dram_tensor("xt", [768, 1024], f32r, kind="ExternalInput").ap()
    wqk = nc.dram_tensor("wqk", [768, 1536], f32r, kind="ExternalInput").ap()
    wv = nc.dram_tensor("wv", [768, 768], f32r, kind="ExternalInput").ap()
    wp = nc.dram_tensor("wp", [768, 768], bf16, kind="ExternalInput").ap()
    bias = nc.dram_tensor("bias", [128, 768], f32, kind="ExternalInput").ap()
    ind = nc.dram_tensor("ind", [64, 1024], bf16, kind="ExternalInput").ap()
    rfh = nc.dram_tensor("rfh", [64, 63], bf16, kind="ExternalInput").ap()
    rfw = nc.dram_tensor("rfw", [64, 63], bf16, kind="ExternalInput").ap()
    y = nc.dram_tensor("y", [1024, 768], f32, kind="ExternalOutput").ap()

    with tile.TileContext(nc) as tc, ExitStack() as es:
        singles = es.enter_context(tc.tile_pool(name="singles", bufs=1))

        # qaug: per head a [128, 1024] aug-rhs block: rows 0:64 = qT (scaled),
        # 64:96 = rel_hT, 96:128 = rel_wT. Heads side by side in columns.
        qaug = singles.tile([128, 12 * 1024], bf16)
        # v in natural layout + ones column per head: [k2-part, chunk, head, 65]
        vaug = singles.tile([128, 8, 12, 65], bf16)
        rfh_sb = singles.tile([64, 63], bf16)
        rfw_sb = singles.tile([64, 63], bf16)
        # Assembled S_T lhsT tiles: rows 0:64 = kT chunk, rows 64:128 =
        # constant indicator rows. Axes: [pair-parity, head-parity, chunk].
        kasm = singles.tile([128, 2, 2, 8, 128], bf16)
        xt_sb = singles.tile([128, 6, 1024], f32r)
        wq_sb = singles.tile([128, 6, 768], f32r)
        wv_sb = singles.tile([128, 6, 768], f32r)
        wp_sb = singles.tile([128, 6, 768], bf16)
        bias_sb = singles.tile([128, 768], f32)
        atile = singles.tile([128, 6, 1024], bf16)  # attn out, [c, n] layout

        xt_r = xt.rearrange("(ko p) n -> p ko n", p=128)
        wqk_r = wqk.rearrange("(ko p) n -> p ko n", p=128)
        wv_r = wv.rearrange("(ko p) n -> p ko n", p=128)
        wp_r = wp.rearrange("(ko p) n -> p ko n", p=128)

        # ---- one-time loads: weights, tables, indicator rows, exp warm ----
        with ExitStack() as es0:
            init = es0.enter_context(tc.tile_pool(name="init", bufs=1))
            ind_sb = init.tile([64, 1024], bf16)
            nc.gpsimd.dma_start(rfh_sb, rfh)
            nc.gpsimd.dma_start(rfw_sb, rfw)
            nc.gpsimd.dma_start(ind_sb, ind)
            for k in range(0, 6, 2):
                nc.sync.dma_start(xt_sb[:, k], xt_r[:, k])
                nc.scalar.dma_start(xt_sb[:, k + 1], xt_r[:, k + 1])
            for k in range(0, 6, 2):
                nc.sync.dma_start(wq_sb[:, k], wqk_r[:, k, 0:768])
                nc.scalar.dma_start(wq_sb[:, k + 1], wqk_r[:, k + 1, 0:768])
            for k in range(6):
                nc.gpsimd.dma_start(wv_sb[:, k], wv_r[:, k])
                nc.gpsimd.dma_start(wp_sb[:, k], wp_r[:, k])
            nc.gpsimd.dma_start(bias_sb, bias)
            for b in range(2):
                for p in range(2):
                    nc.vector.tensor_copy(
                        kasm[64:128, b, p],
                        ind_sb.rearrange("p (c n) -> p c n", c=8),
                    )
            nc.vector.memset(vaug[:, :, :, 64:65], 1.0)
            warm = init.tile([1, 1], f32)
            nc.vector.memset(warm, 0.0)
            nc.scalar.activation(warm, warm, Exp)

        for _rep in range(reps):
          with ExitStack() as esR:
            # ------- Phase B/C: q projection, rel-pos rows, v projection -------
            with ExitStack() as esB:
                bv = esB.enter_context(tc.tile_pool(name="bv", bufs=2, space="PSUM"))
                esQ = esB.enter_context(ExitStack())
                bqk = esQ.enter_context(tc.tile_pool(name="bqk", bufs=2, space="PSUM"))

                # q, transposed layout: out rows = head*64+d, cols = n
                for m in range(6):
                    for n in range(2):
                        ps = bqk.tile([128, 512], f32)
                        for k in range(6):
                            mm(
                                ps,
                                wq_sb[:, k, m * 128 : (m + 1) * 128],
                                xt_sb[:, k, n * 512 : (n + 1) * 512],
                                start=(k == 0),
                                stop=(k == 5),
                            )
                        for half, hd in ((0, 2 * m), (64, 2 * m + 1)):
                            nc.scalar.copy(
                                qaug[0:64, hd * 1024 + n * 512 : hd * 1024 + (n + 1) * 512],
                                ps[half : half + 64, :],
                            )

                # rel-pos rows: rel_hT[k,(head,h,w)] = sum_c rel_pos_h[h-k+31,c]
                # * qT[c,(head,h,w)]; one matmul per h (w) over all heads via
                # the flipped-table slice.  bqk is closed first so cps can
                # have 4 bufs; evac copies alternate ACT/DVE.
                esQ.close()
                cps = esB.enter_context(tc.tile_pool(name="cps", bufs=4, space="PSUM"))
                qaug4d = qaug.rearrange("p (hd a b) -> p hd a b", hd=12, a=32)
                for hh in range(32):
                    pg = cps.tile([32, 12, 32], f32)
                    mm(pg, rfh_sb[:, 31 - hh : 63 - hh], qaug4d[0:64, :, hh, :],
                       start=True, stop=True)
                    if hh % 2 == 0:
                        nc.vector.tensor_copy(qaug4d[64:96, :, hh, :], pg)
                    else:
                        nc.scalar.copy(qaug4d[64:96, :, hh, :], pg)
                for ww in range(32):
                    pg = cps.tile([32, 12, 32], f32)
                    mm(pg, rfw_sb[:, 31 - ww : 63 - ww], qaug4d[0:64, :, :, ww],
                       start=True, stop=True)
                    if ww % 2 == 0:
                        nc.vector.tensor_copy(qaug4d[96:128, :, :, ww], pg)
                    else:
                        nc.scalar.copy(qaug4d[96:128, :, :, ww], pg)

                # v in natural layout [n, c]
                for ch in range(8):
                    pv = bv.tile([128, 768], f32)
                    for c0, cw in ((0, 512), (512, 256)):
                        for k in range(6):
                            mm(
                                pv[:, c0 : c0 + cw],
                                xt_sb[:, k, ch * 128 : (ch + 1) * 128],
                                wv_sb[:, k, c0 : c0 + cw],
                                start=(k == 0),
                                stop=(k == 5),
                            )
                    nc.vector.tensor_copy(
                        vaug[:, ch, :, 0:64], pv.rearrange("p (h d) -> p h d", h=12)
                    )

            # ---------------- Phase D: attention per head ----------------
            expp = esR.enter_context(tc.tile_pool(name="expp", bufs=3))
            unp = esR.enter_context(tc.tile_pool(name="unp", bufs=2))
            recp = esR.enter_context(tc.tile_pool(name="recp", bufs=2))
            wkp = esR.enter_context(tc.tile_pool(name="wkp", bufs=2))
            dps = esR.enter_context(tc.tile_pool(name="dps", bufs=2, space="PSUM"))
            dpo = esR.enter_context(tc.tile_pool(name="dpo", bufs=2, space="PSUM"))
            epool = esR.enter_context(tc.tile_pool(name="epool", bufs=3))

            wkt_t = {}

            def fetch_wk(t):
                wkt = wkp.tile([128, 6, 128], f32r)
                wkt_t[t] = wkt
                for k in range(6):
                    nc.sync.dma_start(
                        wkt[:, k], wqk_r[:, k, 768 + t * 128 : 768 + (t + 1) * 128]
                    )

            def k_group(t):
                # k rows for pair t -> kasm[t % 2][*][*] rows 0:64
                wkt = wkt_t.pop(t)
                kp = [dpo.tile([128, 512], f32, name=f"kp{n}", tag="kp") for n in range(2)]
                for n in range(2):
                    for k in range(6):
                        mm(
                            kp[n],
                            wkt[:, k],
                            xt_sb[:, k, n * 512 : (n + 1) * 512],
                            start=(k == 0),
                            stop=(k == 5),
                        )
                    kp4 = kp[n].rearrange("p (c n2) -> p c n2", c=4)
                    for p in range(2):
                        nc.vector.tensor_copy(
                            kasm[0:64, t % 2, p, 4 * n : 4 * n + 4],
                            kp4[64 * p : 64 * p + 64],
                        )

            fetch_wk(0)
            k_group(0)
            fetch_wk(1)

            o_ps_h = {}
            un_h = {}
            rec_h = {}

            def yblock(ch, klist, stop):
                yp = yp_ch[ch]
                for k in klist:
                    for c0, cw in ((0, 512), (512, 256)):
                        mm(
                            yp[:, c0 : c0 + cw],
                            atile[:, k, ch * 128 : (ch + 1) * 128],
                            wp_sb[:, k, c0 : c0 + cw],
                            start=(k == 0),
                            stop=(stop and k == 5),
                        )

            yp_ch = {}

            for it in range(14):
                # stage A (head=it-1): evacuate o_ps; reciprocal of den row;
                # broadcast the reciprocal over 64 partitions (gpsimd)
                if 0 <= it - 1 < 12:
                    hd = it - 1
                    o_ps = o_ps_h.pop(hd)
                    # rows 0:64 = unnormalized out, row 64 = denominator
                    un = unp.tile([65, 2, 512], f32, name="unnorm")
                    un_h[hd] = un
                    for nt in range(2):
                        nc.vector.tensor_copy(un[:, nt], o_ps[nt])
                    rec = recp.tile([1, 2, 512], f32, tag="rc")
                    nc.vector.reciprocal(rec, un[64:65])
                    rep = recp.tile([64, 2, 512], f32, tag="bc")
                    rec_h[hd] = rep
                    nc.gpsimd.partition_broadcast(rep, rec)

                # stage B (head=it-2): normalize into atile
                if 0 <= it - 2 < 12:
                    hd = it - 2
                    t3 = hd // 2
                    half3 = (hd % 2) * 64
                    rep = rec_h.pop(hd)
                    un = un_h.pop(hd)
                    a3 = atile[half3 : half3 + 64, t3, :].rearrange(
                        "p (a b) -> p a b", a=2
                    )
                    nc.vector.tensor_mul(a3, un[0:64], rep)

                # early phase E: k-blocks 0..4 only need heads 0..9, which are
                # normalized by it=11; fills the denominator-drain PE idle.
                # Only 2 chunks fit the 2 "sps" PSUM slots.
                if it == 12:
                    for ch in range(2):
                        yp_ch[ch] = dps.tile([128, 768], f32, tag="sps", name="yp")
                        yblock(ch, range(5), stop=False)

                # prefetch next rep's xt once the last k_group consumed it
                if it == 10 and _rep + 1 < reps:
                    for k in range(6):
                        nc.sync.dma_start(xt_sb[:, k], xt_r[:, k])

                # stage 0 (head=it): attention chunks (+ next pair's k group)
                if it < 12:
                    hd = it
                    par = hd % 2
                    t = hd // 2
                    if par == 1 and t + 1 < 6:
                        k_group(t + 1)
                        if t + 2 < 6:
                            fetch_wk(t + 2)
                    o_ps = [dpo.tile([65, 512], f32, name=f"ops{nt}", tag="ops") for nt in range(2)]
                    o_ps_h[hd] = o_ps
                    for ch in range(8):
                        s_ps = dps.tile([128, 1024], f32, tag="sps")
                        for nt in range(2):
                            mm(
                                s_ps[:, nt * 512 : (nt + 1) * 512],
                                kasm[:, t % 2, par, ch],
                                qaug[:, hd * 1024 + nt * 512 : hd * 1024 + (nt + 1) * 512],
                                start=True,
                                stop=True,
                            )
                        ex = expp.tile([128, 1024], bf16)
                        nc.scalar.activation(ex, s_ps, Exp)
                        for nt in range(2):
                            mm(
                                o_ps[nt],
                                vaug[:, ch, hd, :],
                                ex[:, nt * 512 : (nt + 1) * 512],
                                start=(ch == 0),
                                stop=(ch == 7),
                            )

            # ---------------- Phase E: output projection (tail) ----------------
            for ch in range(8):
                if ch < 2:
                    yblock(ch, [5], stop=True)
                else:
                    yp_ch[ch] = dps.tile([128, 768], f32, tag="sps", name="yp")
                    yblock(ch, range(6), stop=True)
                yp = yp_ch.pop(ch)
                y_sb = epool.tile([128, 768], f32)
                nc.vector.tensor_add(y_sb, yp, bias_sb)
                nc.sync.dma_start(y[ch * 128 : (ch + 1) * 128, :], y_sb)

    nc.compile()
    return nc


def _host_prep(qkv_w, rel_pos_h, rel_pos_w, proj_w, proj_b):
    import ml_dtypes

    bf16 = ml_dtypes.bfloat16
    qkv_w = np.asarray(qkv_w, np.float32)
    scale = 1.0 / np.sqrt(HD)
    wqk = np.ascontiguousarray(qkv_w[0:1536].T)  # [768, 1536]
    wqk[:, 0:768] *= scale
    wv = np.ascontiguousarray(qkv_w[1536:2304].T)  # [768, 768]
    wp = np.ascontiguousarray(np.asarray(proj_w, np.float32).T).astype(bf16)
    bias = np.ascontiguousarray(
        np.broadcast_to(np.asarray(proj_b, np.float32)[None, :], (128, 768))
    )
    k2 = np.arange(1024)
    indm = np.zeros((64, 1024), np.float32)
    indm[0:32] = (k2[None, :] // 32) == np.arange(32)[:, None]
    indm[32:64] = (k2[None, :] % 32) == np.arange(32)[:, None]
    rfh = np.ascontiguousarray(np.asarray(rel_pos_h, np.float32)[::-1].T).astype(bf16)
    rfw = np.ascontiguousarray(np.asarray(rel_pos_w, np.float32)[::-1].T).astype(bf16)
    return dict(
        wqk=wqk, wv=wv, wp=wp, bias=bias, ind=indm.astype(bf16), rfh=rfh, rfw=rfw
    )


def get_nc(reps=1):
    key = ("nc", reps)
    if key not in _CACHE:
        _CACHE[key] = _build_nc(reps=reps)
    return _CACHE[key]


def make_in_maps(x, qkv_w, rel_pos_h, rel_pos_w, proj_w, proj_b):
    shared = _host_prep(qkv_w, rel_pos_h, rel_pos_w, proj_w, proj_b)
    x = np.asarray(x, np.float32)
    return [
        dict(shared, xt=np.ascontiguousarray(x[b].T)) for b in range(x.shape[0])
    ]


def kernel(x, qkv_w, rel_pos_h, rel_pos_w, proj_w, proj_b, H=32, W=32):
    from concourse.bass_utils import run_bass_kernel_spmd

    nc = get_nc()
    in_maps = make_in_maps(x, qkv_w, rel_pos_h, rel_pos_w, proj_w, proj_b)
    res = run_bass_kernel_spmd(nc, in_maps, list(range(NCORES)))
    out = np.stack([np.asarray(res.results[b]["y"]) for b in range(NCORES)])
    return out.astype(np.float32)
